# revision 25
# baseline (speedup 1.0000x reference)
# Trainium2 Bass kernel for nn_FHNTritonAttention: causal attention with an
# FHN (FitzHugh-Nagumo) gate on the attention probabilities.
#
# Math note that shapes the whole kernel: attn_energy = softmax(scores).sum(-1)
# is ~1.0 for every row (softmax rows sum to 1), so stimulus_normed == 1,
# threshold_gate == sigmoid(5), and the FHN recurrence collapses to one
# constant per run. The gate multiplies each probability row by a constant c
# and the subsequent renormalization divides it back out:
#   p'' = p*c / (c*S + 1e-8) = p / (S + 1e-8/c),  S = row sum ~= 1.
# So the entire FHN block reduces to scaling the output by
# f0 = 1/(1 + 1e-8/c0), computed on host from (a, b, dt) and folded into Wout.
# The deviations this ignores are O(1e-7) relative — far below fp32 matmul
# noise of the reference itself.
#
# Device kernel (SPMD over 8 cores; core = (batch, 4-head group)):
#   phase A: qkT = Wqk_slice @ x.T   (transposed layout: feature on partitions)
#            v_nat = x @ Wv_slice.T  (natural layout, + ones column for Z)
#   phase B: per head PAIR (two heads side by side in one 128-partition tile),
#            per 512-wide q tile, per 128-deep s chunk:
#            scoresT = k q^T (s on partitions) for both heads into one
#            [128, 1024] psum (2 banks), one exp -> U bf16, one causal-mask
#            multiply on diagonal chunks (mask doubled per head),
#            PV: [v | 1]^T @ U accumulates outT and the softmax denominator Z
#            in one matmul per head. pv psum is evicted by two quick copies
#            (outT -> attn tile, Z row -> zbuf); reciprocals are batched per
#            pair (one DVE reciprocal for 8 rows), replication of 1/Z across
#            64 partitions via a 1-partition matmul, normalize multiplies
#            in-place against the replication PSUM.
#   phase C: yT_partial = Wout_slice @ attn_outT  -> DMA out; host sums the 4
#            partial products per batch and transposes back.
#
# Matmuls run in bf16 (inputs pre-cast on host; fp32 PSUM accumulation), the
# 1/Z path in float32r.

import math
import os

import numpy as np

N_HEADS = 16
HEAD_DIM = 64
THRESHOLD = 0.5
TAU = 12.5
N_FHN_STEPS = 2

N_CORES = 8
HEADS_PER_CORE = 4  # cores 0-3 -> batch 0, cores 4-7 -> batch 1

ATTN_DTYPE = os.environ.get("KERNEL_ATTN_DTYPE", "bfloat16")

LAST_RUN = {}  # filled with exec_time_ns etc. when KERNEL_TRACE is set

_PROGRAM_CACHE = {}


def _fhn_scale(a, b, dt):
    """Host-side replica of the reference's gate math at attn_energy == 1."""
    a = float(a)
    b = float(b)
    dt = float(dt)
    sig5 = 1.0 / (1.0 + math.exp(-(1.0 - THRESHOLD) * 10.0))
    i0 = 1.0 * (0.1 + 0.9 * sig5)
    v = 0.0
    w = 0.0
    for _ in range(N_FHN_STEPS):
        v = v + dt * (v - v**3 / 3.0 - w + i0)
        w = (w + (dt / TAU) * (v + a)) / (1.0 + (dt / TAU) * b)
    gate = 1.0 / (1.0 + math.exp(-v))
    c0 = 0.5 + 0.5 * gate
    return c0 / (c0 + 1e-8)


def _build_program(T, D, H_per_core, hd):
    import concourse.mybir as mybir
    import concourse.tile as tile
    from concourse import bacc

    f32 = mybir.dt.float32
    at_dt = getattr(mybir.dt, ATTN_DTYPE)
    P = 128
    QT = 512   # q tile width (free dim of score/PV matmuls)
    SC = 128   # s chunk depth (contraction of PV, partitions of scoresT)
    K_D = D // P
    QK_ROWS = 2 * H_per_core * hd
    V_COLS = H_per_core * hd
    C = H_per_core * hd
    n_qt = T // QT
    n_pairs = H_per_core // 2

    nc = bacc.Bacc("TRN2", target_bir_lowering=False, debug=False,
                   num_devices=N_CORES)

    xt_d = nc.declare_dram_parameter("xt", [D, T], at_dt, isOutput=False)
    wqkt_d = nc.declare_dram_parameter("wqkt", [D, QK_ROWS], at_dt, isOutput=False)
    wvt_d = nc.declare_dram_parameter("wvt", [D, V_COLS], at_dt, isOutput=False)
    wot_d = nc.declare_dram_parameter("wot", [C, D], at_dt, isOutput=False)
    masks_d = nc.declare_dram_parameter("masks", [P, 4 * 2 * QT], at_dt,
                                        isOutput=False)
    yt_d = nc.declare_dram_parameter("yt", [D, T], f32, isOutput=True)

    xt_t = xt_d.rearrange("(a p) t -> a p t", p=P)
    wqkt_t = wqkt_d.rearrange("(a p) m -> a p m", p=P)
    wvt_t = wvt_d.rearrange("(a p) m -> a p m", p=P)
    wot_t = wot_d.rearrange("(a p) m -> a p m", p=P)
    yt_t = yt_d.rearrange("(a p) t -> a p t", p=P)

    with nc.allow_low_precision(reason="bf16/f32r compute is intentional"), \
            tile.TileContext(nc) as tc:
        with (
            tc.tile_pool(name="persist", bufs=1) as persist,
            tc.tile_pool(name="xw", bufs=1) as xw,
            tc.tile_pool(name="aps_pool", bufs=2, space="PSUM") as aps_pool,
            tc.tile_pool(name="sc_ps", bufs=2, space="PSUM") as sc_ps,
            tc.tile_pool(name="pv_ps", bufs=1, space="PSUM") as pv_ps,
            tc.tile_pool(name="u_sb", bufs=6) as u_pool,
            tc.tile_pool(name="norm", bufs=1) as norm_pool,
            tc.tile_pool(name="y_sb", bufs=2) as y_pool,
        ):
            # ---- input DMAs ----
            xt = [xw.tile([P, T], at_dt, name=f"xt{i}", tag=f"xt{i}")
                  for i in range(K_D)]
            wqkt = [xw.tile([P, QK_ROWS], at_dt, name=f"wqkt{i}", tag=f"wqkt{i}")
                    for i in range(K_D)]
            wvt = [xw.tile([P, V_COLS], at_dt, name=f"wvt{i}", tag=f"wvt{i}")
                   for i in range(K_D)]
            for i in range(K_D):
                nc.sync.dma_start(out=wqkt[i][:], in_=wqkt_t[i])
            # xt arrives in T-chunks, n-major, so the first QKV psum group can
            # start as soon as the weights + 1/4 of xt have landed
            for i in range(K_D):
                nc.sync.dma_start(out=xt[i][:, 0:512], in_=xt_t[i][:, 0:512])
            for i in range(K_D):
                nc.sync.dma_start(out=wvt[i][:], in_=wvt_t[i])
            for n in range(1, T // 512):
                for i in range(K_D):
                    nc.sync.dma_start(out=xt[i][:, n * 512:(n + 1) * 512],
                                      in_=xt_t[i][:, n * 512:(n + 1) * 512])
            masks = persist.tile([P, 8 * QT], at_dt, name="masks", tag="masks")
            nc.sync.dma_start(out=masks[:], in_=masks_d[:])
            wot = [persist.tile([P, D], at_dt, name=f"wot{i}", tag=f"wot{i}")
                   for i in range(C // P)]
            for i in range(C // P):
                nc.sync.dma_start(out=wot[i][:], in_=wot_t[i])

            ones_f32 = persist.tile([P, hd], f32, name="ones_f32", tag="ones_f32")
            nc.vector.memset(ones_f32[:], 1.0)

            # ---- phase A: qkT (transposed) + v (natural), n-chunk-major ----
            qkt = [persist.tile([P, T], at_dt, name=f"qkt{m}", tag=f"qkt{m}")
                   for m in range(QK_ROWS // P)]
            v_pad = [persist.tile([P, H_per_core * (hd + 1)], at_dt,
                                  name=f"vp{m}", tag=f"vp{m}")
                     for m in range(T // P)]
            for n in range(T // 512):
                for m in range(QK_ROWS // P):
                    ps = aps_pool.tile([P, 512], f32, name="qkps", tag="aps")
                    for k in range(K_D):
                        nc.tensor.matmul(
                            ps[:],
                            lhsT=wqkt[k][:, m * P:(m + 1) * P],
                            rhs=xt[k][:, n * 512:(n + 1) * 512],
                            start=(k == 0), stop=(k == K_D - 1),
                        )
                    nc.vector.tensor_copy(qkt[m][:, n * 512:(n + 1) * 512], ps[:])
                for m in range(4 * n, 4 * n + 4):
                    ones_cols = v_pad[m].rearrange(
                        "p (h x) -> p h x", x=hd + 1)[:, :, hd:]
                    nc.vector.tensor_copy(
                        ones_cols,
                        ones_f32[:, 0:H_per_core].rearrange("p (h x) -> p h x", x=1))
                    ps = aps_pool.tile([P, V_COLS], f32, name="vps", tag="aps")
                    for k in range(K_D):
                        nc.tensor.matmul(
                            ps[:],
                            lhsT=xt[k][:, m * P:(m + 1) * P],
                            rhs=wvt[k][:],
                            start=(k == 0), stop=(k == K_D - 1),
                        )
                    for h in range(H_per_core):
                        nc.vector.tensor_copy(
                            v_pad[m][:, h * (hd + 1):h * (hd + 1) + hd],
                            ps[:, h * hd:(h + 1) * hd],
                        )

            # ---- phase B: attention (g-outer) + interleaved out-projection ----
            attn = [persist.tile([P, T], at_dt, name=f"attn{p}", tag=f"attn{p}")
                    for p in range(n_pairs)]
            for g in range(n_qt):
                for p in range(n_pairs):
                    qT = qkt[p]        # heads (2p, 2p+1) on partitions 0:64, 64:128
                    kT = qkt[n_pairs + p]
                    q0 = g * QT
                    n_sc = (q0 + QT) // SC
                    pv = [pv_ps.tile([hd + 1, QT], f32, name=f"pv{e}", tag=f"pv{e}")
                          for e in range(2)]
                    for j in range(n_sc):
                        s0 = j * SC
                        sc = sc_ps.tile([P, 2 * QT], f32, name="sc", tag="sc")
                        for e in range(2):
                            lo, hi = e * 64, e * 64 + 64
                            nc.tensor.matmul(
                                sc[:, e * QT:(e + 1) * QT],
                                lhsT=kT[lo:hi, s0:s0 + SC],
                                rhs=qT[lo:hi, q0:q0 + QT],
                                start=True, stop=True,
                            )
                        u = u_pool.tile([P, 2 * QT], at_dt, name="u", tag="u")
                        nc.scalar.activation(
                            u[:], sc[:], mybir.ActivationFunctionType.Exp,
                            scale=1.0 / math.sqrt(hd),
                        )
                        r = (s0 - q0) // SC
                        if r >= 0:  # diagonal chunk: causal mask on idle GpSimd
                            for e in range(2):
                                off = e * QT
                                if r > 0:  # fully-invalid strip -> zero
                                    nc.gpsimd.memset(
                                        u[:, off:off + SC * r], 0.0)
                                tri = r * 2 * QT + e * QT + SC * r
                                nc.gpsimd.tensor_mul(
                                    u[:, off + SC * r:off + SC * r + SC],
                                    u[:, off + SC * r:off + SC * r + SC],
                                    masks[:, tri:tri + SC])
                        for e in range(2):
                            h = 2 * p + e
                            nc.tensor.matmul(
                                pv[e][:],
                                lhsT=v_pad[j][:, h * (hd + 1):(h + 1) * (hd + 1)],
                                rhs=u[:, e * QT:(e + 1) * QT],
                                start=(j == 0), stop=(j == n_sc - 1),
                            )
                    reps = []
                    for e in range(2):
                        # evict pv bank: unnormalized outT + Z row
                        nc.vector.tensor_copy(
                            attn[p][e * hd:(e + 1) * hd, q0:q0 + QT],
                            pv[e][0:hd, :])
                        zrow = norm_pool.tile([1, QT], f32, name="zrow",
                                              tag=f"zrow{e}", bufs=4)
                        nc.vector.tensor_copy(zrow[0:1, :], pv[e][hd:hd + 1, :])
                        rrow = norm_pool.tile([1, QT], f32, name="rrow",
                                              tag=f"rrow{e}", bufs=4)
                        nc.vector.reciprocal_approx_fast(
                            out=rrow[0:1, :], in_=zrow[0:1, :])
                        rep = norm_pool.tile([P, QT], f32, name="rep",
                                             tag=f"rep{e}", bufs=4)
                        nc.gpsimd.partition_broadcast(rep[:], rrow[0:1, :])
                        reps.append(rep)
                    for e in range(2):
                        sl = attn[p][e * hd:(e + 1) * hd, q0:q0 + QT]
                        nc.vector.tensor_mul(
                            sl, sl, reps[e][e * hd:(e + 1) * hd, :])

                # out-projection for this q-tile (t columns g*QT..): both pairs
                # of heads are normalized now, so contract all C rows
                for m in range(D // P):
                    ps = aps_pool.tile([P, QT], f32, name="yps", tag="aps")
                    for k in range(C // P):
                        nc.tensor.matmul(
                            ps[:],
                            lhsT=wot[k][:, m * P:(m + 1) * P],
                            rhs=attn[k][:, g * QT:(g + 1) * QT],
                            start=(k == 0), stop=(k == C // P - 1),
                        )
                    y = y_pool.tile([P, QT], f32, name="y", tag="y", bufs=4)
                    nc.vector.tensor_copy(y[:], ps[:])
                    nc.sync.dma_start(
                        out=yt_t[m][:, g * QT:(g + 1) * QT], in_=y[:])

    nc.finalize()
    return nc


def _make_masks(QT=512, SC=128):
    """Doubled causal masks: [128, 4*2*QT]; block r holds the mask for
    relative offset r twice side by side (head A | head B)."""
    i = np.arange(SC)[:, None]
    j = np.arange(QT)[None, :]
    blocks = []
    for r in range(4):
        m = (i + r * SC <= j).astype(np.float32)
        blocks += [m, m]
    return np.concatenate(blocks, axis=1)


def _cast(arr, dtype_name):
    if dtype_name == "bfloat16":
        import ml_dtypes
        return np.ascontiguousarray(arr.astype(ml_dtypes.bfloat16))
    return np.ascontiguousarray(arr.astype(np.float32))


def kernel(x, Wqkv, Wout, a, b, dt):
    from concourse.bass_utils import run_bass_kernel_spmd

    x = np.asarray(x, dtype=np.float32)
    Wqkv = np.asarray(Wqkv, dtype=np.float32)
    Wout = np.asarray(Wout, dtype=np.float32)
    B, T, D = x.shape
    H, hd = N_HEADS, HEAD_DIM
    hpc = HEADS_PER_CORE
    cores_per_batch = H // hpc
    f0 = _fhn_scale(a, b, dt)

    key = (T, D, hpc, hd)
    if key not in _PROGRAM_CACHE:
        _PROGRAM_CACHE[key] = _build_program(*key)
    nc = _PROGRAM_CACHE[key]

    masks = _cast(_make_masks(), ATTN_DTYPE)
    in_maps = []
    for c in range(N_CORES):
        bi = c // cores_per_batch
        heads = range((c % cores_per_batch) * hpc, (c % cores_per_batch) * hpc + hpc)
        q_rows = np.concatenate([np.arange(h * hd, (h + 1) * hd) for h in heads])
        xt = _cast(x[bi].T, ATTN_DTYPE)                          # (D, T)
        wqk = np.concatenate([Wqkv[q_rows], Wqkv[D + q_rows]], axis=0)
        wqkt = _cast(wqk.T, ATTN_DTYPE)                          # (D, 2*hpc*hd)
        wvt = _cast(Wqkv[2 * D + q_rows].T, ATTN_DTYPE)          # (D, hpc*hd)
        wo = (Wout[:, q_rows].astype(np.float64) * f0).astype(np.float32)
        wot = _cast(wo.T, ATTN_DTYPE)                            # (hpc*hd, D)
        in_maps.append({"xt": xt, "wqkt": wqkt, "wvt": wvt, "wot": wot,
                        "masks": masks})

    trace_dir = os.environ.get("KERNEL_TRACE", "")
    kwargs = {}
    if trace_dir:
        os.makedirs(trace_dir, exist_ok=True)
        kwargs = {"trace": True, "tmpdir": trace_dir}
    res = run_bass_kernel_spmd(nc, in_maps, list(range(N_CORES)), **kwargs)
    LAST_RUN["exec_time_ns"] = res.exec_time_ns
    LAST_RUN["profile_json"] = res.profile_json

    out = np.zeros((B, T, D), dtype=np.float32)
    for bi in range(B):
        acc = np.zeros((D, T), dtype=np.float32)
        for c in range(bi * cores_per_batch, (bi + 1) * cores_per_batch):
            acc += res.results[c]["yt"]
        out[bi] = acc.T
    return out


# revision 26
# speedup vs baseline: 1.5222x; 1.5222x over previous
# Trainium2 Bass kernel for nn_FHNTritonAttention: causal attention with an
# FHN (FitzHugh-Nagumo) gate on the attention probabilities.
#
# Math note that shapes the whole kernel: attn_energy = softmax(scores).sum(-1)
# is ~1.0 for every row (softmax rows sum to 1), so stimulus_normed == 1,
# threshold_gate == sigmoid(5), and the FHN recurrence collapses to one
# constant per run. The gate multiplies each probability row by a constant c
# and the subsequent renormalization divides it back out:
#   p'' = p*c / (c*S + 1e-8) = p / (S + 1e-8/c),  S = row sum ~= 1.
# So the entire FHN block reduces to scaling the output by
# f0 = 1/(1 + 1e-8/c0), computed on host from (a, b, dt) and folded into Wout.
# The deviations this ignores are O(1e-7) relative — far below fp32 matmul
# noise of the reference itself.
#
# Device kernel (SPMD over 8 cores; core = (batch, 4-head group)):
#   phase A: qkT = Wqk_slice @ x.T   (transposed layout: feature on partitions)
#            v_nat = x @ Wv_slice.T  (natural layout, + ones column for Z)
#   phase B: per head PAIR (two heads side by side in one 128-partition tile),
#            per 512-wide q tile, per 128-deep s chunk:
#            scoresT = k q^T (s on partitions) for both heads into one
#            [128, 1024] psum (2 banks), one exp -> U bf16, one causal-mask
#            multiply on diagonal chunks (mask doubled per head),
#            PV: [v | 1]^T @ U accumulates outT and the softmax denominator Z
#            in one matmul per head. pv psum is evicted by two quick copies
#            (outT -> attn tile, Z row -> zbuf); reciprocals are batched per
#            pair (one DVE reciprocal for 8 rows), replication of 1/Z across
#            64 partitions via a 1-partition matmul, normalize multiplies
#            in-place against the replication PSUM.
#   phase C: yT_partial = Wout_slice @ attn_outT  -> DMA out; host sums the 4
#            partial products per batch and transposes back.
#
# Matmuls run in bf16 (inputs pre-cast on host; fp32 PSUM accumulation), the
# 1/Z path in float32r.

import math
import os

import numpy as np

N_HEADS = 16
HEAD_DIM = 64
THRESHOLD = 0.5
TAU = 12.5
N_FHN_STEPS = 2

N_CORES = 8
HEADS_PER_CORE = 4  # cores 0-3 -> batch 0, cores 4-7 -> batch 1

ATTN_DTYPE = os.environ.get("KERNEL_ATTN_DTYPE", "bfloat16")

LAST_RUN = {}  # filled with exec_time_ns etc. when KERNEL_TRACE is set

_PROGRAM_CACHE = {}


def _fhn_scale(a, b, dt):
    """Host-side replica of the reference's gate math at attn_energy == 1."""
    a = float(a)
    b = float(b)
    dt = float(dt)
    sig5 = 1.0 / (1.0 + math.exp(-(1.0 - THRESHOLD) * 10.0))
    i0 = 1.0 * (0.1 + 0.9 * sig5)
    v = 0.0
    w = 0.0
    for _ in range(N_FHN_STEPS):
        v = v + dt * (v - v**3 / 3.0 - w + i0)
        w = (w + (dt / TAU) * (v + a)) / (1.0 + (dt / TAU) * b)
    gate = 1.0 / (1.0 + math.exp(-v))
    c0 = 0.5 + 0.5 * gate
    return c0 / (c0 + 1e-8)


def _build_program(T, D, H_per_core, hd):
    import concourse.mybir as mybir
    import concourse.tile as tile
    from concourse import bacc

    f32 = mybir.dt.float32
    at_dt = getattr(mybir.dt, ATTN_DTYPE)
    P = 128
    QT = 512   # q tile width (free dim of score/PV matmuls)
    SC = 128   # s chunk depth (contraction of PV, partitions of scoresT)
    K_D = D // P
    QK_ROWS = 2 * H_per_core * hd
    V_COLS = H_per_core * hd
    C = H_per_core * hd
    n_qt = T // QT
    n_pairs = H_per_core // 2

    nc = bacc.Bacc("TRN2", target_bir_lowering=False, debug=False,
                   num_devices=N_CORES)

    xt_d = nc.declare_dram_parameter("xt", [D, T], at_dt, isOutput=False)
    wqkt_d = nc.declare_dram_parameter("wqkt", [D, QK_ROWS], at_dt, isOutput=False)
    wvt_d = nc.declare_dram_parameter("wvt", [D, V_COLS], at_dt, isOutput=False)
    wot_d = nc.declare_dram_parameter("wot", [C, D], at_dt, isOutput=False)
    masks_d = nc.declare_dram_parameter("masks", [P, 4 * 2 * QT], at_dt,
                                        isOutput=False)
    yt_d = nc.declare_dram_parameter("yt", [D, T], f32, isOutput=True)

    xt_t = xt_d.rearrange("(a p) t -> a p t", p=P)
    wqkt_t = wqkt_d.rearrange("(a p) m -> a p m", p=P)
    wvt_t = wvt_d.rearrange("(a p) m -> a p m", p=P)
    wot_t = wot_d.rearrange("(a p) m -> a p m", p=P)
    yt_t = yt_d.rearrange("(a p) t -> a p t", p=P)

    with nc.allow_low_precision(reason="bf16/f32r compute is intentional"), \
            tile.TileContext(nc) as tc:
        with (
            tc.tile_pool(name="persist", bufs=1) as persist,
            tc.tile_pool(name="xw", bufs=1) as xw,
            tc.tile_pool(name="aps_pool", bufs=2, space="PSUM") as aps_pool,
            tc.tile_pool(name="sc_ps", bufs=2, space="PSUM") as sc_ps,
            tc.tile_pool(name="pv_ps", bufs=1, space="PSUM") as pv_ps,
            tc.tile_pool(name="u_sb", bufs=6) as u_pool,
            tc.tile_pool(name="norm", bufs=1) as norm_pool,
            tc.tile_pool(name="y_sb", bufs=2) as y_pool,
        ):
            # ---- input DMAs ----
            xt = [xw.tile([P, T], at_dt, name=f"xt{i}", tag=f"xt{i}")
                  for i in range(K_D)]
            wqkt = [xw.tile([P, QK_ROWS], at_dt, name=f"wqkt{i}", tag=f"wqkt{i}")
                    for i in range(K_D)]
            wvt = [xw.tile([P, V_COLS], at_dt, name=f"wvt{i}", tag=f"wvt{i}")
                   for i in range(K_D)]
            for i in range(K_D):
                nc.sync.dma_start(out=wqkt[i][:], in_=wqkt_t[i])
            # xt arrives in T-chunks, n-major, so the first QKV psum group can
            # start as soon as the weights + 1/4 of xt have landed
            for i in range(K_D):
                nc.sync.dma_start(out=xt[i][:, 0:512], in_=xt_t[i][:, 0:512])
            for i in range(K_D):
                nc.sync.dma_start(out=wvt[i][:], in_=wvt_t[i])
            for n in range(1, T // 512):
                for i in range(K_D):
                    nc.sync.dma_start(out=xt[i][:, n * 512:(n + 1) * 512],
                                      in_=xt_t[i][:, n * 512:(n + 1) * 512])
            masks = persist.tile([P, 8 * QT], at_dt, name="masks", tag="masks")
            nc.sync.dma_start(out=masks[:], in_=masks_d[:])
            wot = [persist.tile([P, D], at_dt, name=f"wot{i}", tag=f"wot{i}")
                   for i in range(C // P)]
            for i in range(C // P):
                nc.sync.dma_start(out=wot[i][:], in_=wot_t[i])

            ones_f32 = persist.tile([P, hd], f32, name="ones_f32", tag="ones_f32")
            nc.vector.memset(ones_f32[:], 1.0)

            # ---- phase A: qkT (transposed) + v (natural), n-chunk-major ----
            qkt = [persist.tile([P, T], at_dt, name=f"qkt{m}", tag=f"qkt{m}")
                   for m in range(QK_ROWS // P)]
            v_pad = [persist.tile([P, H_per_core * (hd + 1)], at_dt,
                                  name=f"vp{m}", tag=f"vp{m}")
                     for m in range(T // P)]
            for n in range(T // 512):
                for m in range(QK_ROWS // P):
                    ps = aps_pool.tile([P, 512], f32, name="qkps", tag="aps")
                    for k in range(K_D):
                        nc.tensor.matmul(
                            ps[:],
                            lhsT=wqkt[k][:, m * P:(m + 1) * P],
                            rhs=xt[k][:, n * 512:(n + 1) * 512],
                            start=(k == 0), stop=(k == K_D - 1),
                        )
                    nc.vector.tensor_copy(qkt[m][:, n * 512:(n + 1) * 512], ps[:])
                for m in range(4 * n, 4 * n + 4):
                    ones_cols = v_pad[m].rearrange(
                        "p (h x) -> p h x", x=hd + 1)[:, :, hd:]
                    nc.vector.tensor_copy(
                        ones_cols,
                        ones_f32[:, 0:H_per_core].rearrange("p (h x) -> p h x", x=1))
                    ps = aps_pool.tile([P, V_COLS], f32, name="vps", tag="aps")
                    for k in range(K_D):
                        nc.tensor.matmul(
                            ps[:],
                            lhsT=xt[k][:, m * P:(m + 1) * P],
                            rhs=wvt[k][:],
                            start=(k == 0), stop=(k == K_D - 1),
                        )
                    for h in range(H_per_core):
                        nc.vector.tensor_copy(
                            v_pad[m][:, h * (hd + 1):h * (hd + 1) + hd],
                            ps[:, h * hd:(h + 1) * hd],
                        )

            # ---- phase B: attention (g-outer) + interleaved out-projection ----
            attn = [persist.tile([P, T], at_dt, name=f"attn{p}", tag=f"attn{p}")
                    for p in range(n_pairs)]
            for g in range(n_qt):
                for p in range(n_pairs):
                    qT = qkt[p]        # heads (2p, 2p+1) on partitions 0:64, 64:128
                    kT = qkt[n_pairs + p]
                    q0 = g * QT
                    n_sc = (q0 + QT) // SC
                    pv = [pv_ps.tile([hd + 1, QT], f32, name=f"pv{e}", tag=f"pv{e}")
                          for e in range(2)]
                    for j in range(n_sc):
                        s0 = j * SC
                        sc = sc_ps.tile([P, 2 * QT], f32, name="sc", tag="sc")
                        for e in range(2):
                            lo, hi = e * 64, e * 64 + 64
                            nc.tensor.matmul(
                                sc[:, e * QT:(e + 1) * QT],
                                lhsT=kT[lo:hi, s0:s0 + SC],
                                rhs=qT[lo:hi, q0:q0 + QT],
                                start=True, stop=True,
                            )
                        u = u_pool.tile([P, 2 * QT], at_dt, name="u", tag="u")
                        nc.scalar.activation(
                            u[:], sc[:], mybir.ActivationFunctionType.Exp,
                            scale=1.0 / math.sqrt(hd),
                        )
                        r = (s0 - q0) // SC
                        if r >= 0:  # diagonal chunk: zero invalid strip, then
                            # mask only the 128-wide triangle block per head
                            for e in range(2):
                                off = e * QT
                                if r > 0:
                                    nc.vector.memset(u[:, off:off + SC * r], 0.0)
                                tri = r * 2 * QT + e * QT + SC * r
                                nc.vector.tensor_mul(
                                    u[:, off + SC * r:off + SC * r + SC],
                                    u[:, off + SC * r:off + SC * r + SC],
                                    masks[:, tri:tri + SC])
                        for e in range(2):
                            h = 2 * p + e
                            nc.tensor.matmul(
                                pv[e][:],
                                lhsT=v_pad[j][:, h * (hd + 1):(h + 1) * (hd + 1)],
                                rhs=u[:, e * QT:(e + 1) * QT],
                                start=(j == 0), stop=(j == n_sc - 1),
                            )
                    reps = []
                    for e in range(2):
                        # evict pv bank: unnormalized outT + Z row
                        nc.vector.tensor_copy(
                            attn[p][e * hd:(e + 1) * hd, q0:q0 + QT],
                            pv[e][0:hd, :])
                        zrow = norm_pool.tile([1, QT], f32, name="zrow",
                                              tag=f"zrow{e}", bufs=4)
                        nc.vector.tensor_copy(zrow[0:1, :], pv[e][hd:hd + 1, :])
                        rrow = norm_pool.tile([1, QT], f32, name="rrow",
                                              tag=f"rrow{e}", bufs=4)
                        nc.vector.reciprocal_approx_fast(
                            out=rrow[0:1, :], in_=zrow[0:1, :])
                        rep = norm_pool.tile([P, QT], f32, name="rep",
                                             tag=f"rep{e}", bufs=4)
                        nc.gpsimd.partition_broadcast(rep[:], rrow[0:1, :])
                        reps.append(rep)
                    for e in range(2):
                        sl = attn[p][e * hd:(e + 1) * hd, q0:q0 + QT]
                        nc.vector.tensor_mul(
                            sl, sl, reps[e][e * hd:(e + 1) * hd, :])

                # out-projection for this q-tile (t columns g*QT..): both pairs
                # of heads are normalized now, so contract all C rows
                for m in range(D // P):
                    ps = aps_pool.tile([P, QT], f32, name="yps", tag="aps")
                    for k in range(C // P):
                        nc.tensor.matmul(
                            ps[:],
                            lhsT=wot[k][:, m * P:(m + 1) * P],
                            rhs=attn[k][:, g * QT:(g + 1) * QT],
                            start=(k == 0), stop=(k == C // P - 1),
                        )
                    y = y_pool.tile([P, QT], f32, name="y", tag="y", bufs=4)
                    nc.vector.tensor_copy(y[:], ps[:])
                    nc.sync.dma_start(
                        out=yt_t[m][:, g * QT:(g + 1) * QT], in_=y[:])

    nc.finalize()
    return nc


def _make_masks(QT=512, SC=128):
    """Doubled causal masks: [128, 4*2*QT]; block r holds the mask for
    relative offset r twice side by side (head A | head B)."""
    i = np.arange(SC)[:, None]
    j = np.arange(QT)[None, :]
    blocks = []
    for r in range(4):
        m = (i + r * SC <= j).astype(np.float32)
        blocks += [m, m]
    return np.concatenate(blocks, axis=1)


def _cast(arr, dtype_name):
    if dtype_name == "bfloat16":
        import ml_dtypes
        return np.ascontiguousarray(arr.astype(ml_dtypes.bfloat16))
    return np.ascontiguousarray(arr.astype(np.float32))


def kernel(x, Wqkv, Wout, a, b, dt):
    from concourse.bass_utils import run_bass_kernel_spmd

    x = np.asarray(x, dtype=np.float32)
    Wqkv = np.asarray(Wqkv, dtype=np.float32)
    Wout = np.asarray(Wout, dtype=np.float32)
    B, T, D = x.shape
    H, hd = N_HEADS, HEAD_DIM
    hpc = HEADS_PER_CORE
    cores_per_batch = H // hpc
    f0 = _fhn_scale(a, b, dt)

    key = (T, D, hpc, hd)
    if key not in _PROGRAM_CACHE:
        _PROGRAM_CACHE[key] = _build_program(*key)
    nc = _PROGRAM_CACHE[key]

    masks = _cast(_make_masks(), ATTN_DTYPE)
    in_maps = []
    for c in range(N_CORES):
        bi = c // cores_per_batch
        heads = range((c % cores_per_batch) * hpc, (c % cores_per_batch) * hpc + hpc)
        q_rows = np.concatenate([np.arange(h * hd, (h + 1) * hd) for h in heads])
        xt = _cast(x[bi].T, ATTN_DTYPE)                          # (D, T)
        wqk = np.concatenate([Wqkv[q_rows], Wqkv[D + q_rows]], axis=0)
        wqkt = _cast(wqk.T, ATTN_DTYPE)                          # (D, 2*hpc*hd)
        wvt = _cast(Wqkv[2 * D + q_rows].T, ATTN_DTYPE)          # (D, hpc*hd)
        wo = (Wout[:, q_rows].astype(np.float64) * f0).astype(np.float32)
        wot = _cast(wo.T, ATTN_DTYPE)                            # (hpc*hd, D)
        in_maps.append({"xt": xt, "wqkt": wqkt, "wvt": wvt, "wot": wot,
                        "masks": masks})

    trace_dir = os.environ.get("KERNEL_TRACE", "")
    kwargs = {}
    if trace_dir:
        os.makedirs(trace_dir, exist_ok=True)
        kwargs = {"trace": True, "tmpdir": trace_dir}
    res = run_bass_kernel_spmd(nc, in_maps, list(range(N_CORES)), **kwargs)
    LAST_RUN["exec_time_ns"] = res.exec_time_ns
    LAST_RUN["profile_json"] = res.profile_json

    out = np.zeros((B, T, D), dtype=np.float32)
    for bi in range(B):
        acc = np.zeros((D, T), dtype=np.float32)
        for c in range(bi * cores_per_batch, (bi + 1) * cores_per_batch):
            acc += res.results[c]["yt"]
        out[bi] = acc.T
    return out


# revision 27
# speedup vs baseline: 1.6193x; 1.0638x over previous
# Trainium2 Bass kernel for nn_FHNTritonAttention: causal attention with an
# FHN (FitzHugh-Nagumo) gate on the attention probabilities.
#
# Math note that shapes the whole kernel: attn_energy = softmax(scores).sum(-1)
# is ~1.0 for every row (softmax rows sum to 1), so stimulus_normed == 1,
# threshold_gate == sigmoid(5), and the FHN recurrence collapses to one
# constant per run. The gate multiplies each probability row by a constant c
# and the subsequent renormalization divides it back out:
#   p'' = p*c / (c*S + 1e-8) = p / (S + 1e-8/c),  S = row sum ~= 1.
# So the entire FHN block reduces to scaling the output by
# f0 = 1/(1 + 1e-8/c0), computed on host from (a, b, dt) and folded into Wout.
# The deviations this ignores are O(1e-7) relative — far below fp32 matmul
# noise of the reference itself.
#
# Device kernel (SPMD over 8 cores; core = (batch, 4-head group)):
#   phase A: qkT = Wqk_slice @ x.T   (transposed layout: feature on partitions)
#            v_nat = x @ Wv_slice.T  (natural layout, + ones column for Z)
#   phase B: per head PAIR (two heads side by side in one 128-partition tile),
#            per 512-wide q tile, per 128-deep s chunk:
#            scoresT = k q^T (s on partitions) for both heads into one
#            [128, 1024] psum (2 banks), one exp -> U bf16, one causal-mask
#            multiply on diagonal chunks (mask doubled per head),
#            PV: [v | 1]^T @ U accumulates outT and the softmax denominator Z
#            in one matmul per head. pv psum is evicted by two quick copies
#            (outT -> attn tile, Z row -> zbuf); reciprocals are batched per
#            pair (one DVE reciprocal for 8 rows), replication of 1/Z across
#            64 partitions via a 1-partition matmul, normalize multiplies
#            in-place against the replication PSUM.
#   phase C: yT_partial = Wout_slice @ attn_outT  -> DMA out; host sums the 4
#            partial products per batch and transposes back.
#
# Matmuls run in bf16 (inputs pre-cast on host; fp32 PSUM accumulation), the
# 1/Z path in float32r.

import math
import os

import numpy as np

N_HEADS = 16
HEAD_DIM = 64
THRESHOLD = 0.5
TAU = 12.5
N_FHN_STEPS = 2

N_CORES = 8
HEADS_PER_CORE = 4  # cores 0-3 -> batch 0, cores 4-7 -> batch 1

ATTN_DTYPE = os.environ.get("KERNEL_ATTN_DTYPE", "bfloat16")

LAST_RUN = {}  # filled with exec_time_ns etc. when KERNEL_TRACE is set

_PROGRAM_CACHE = {}


def _fhn_scale(a, b, dt):
    """Host-side replica of the reference's gate math at attn_energy == 1."""
    a = float(a)
    b = float(b)
    dt = float(dt)
    sig5 = 1.0 / (1.0 + math.exp(-(1.0 - THRESHOLD) * 10.0))
    i0 = 1.0 * (0.1 + 0.9 * sig5)
    v = 0.0
    w = 0.0
    for _ in range(N_FHN_STEPS):
        v = v + dt * (v - v**3 / 3.0 - w + i0)
        w = (w + (dt / TAU) * (v + a)) / (1.0 + (dt / TAU) * b)
    gate = 1.0 / (1.0 + math.exp(-v))
    c0 = 0.5 + 0.5 * gate
    return c0 / (c0 + 1e-8)


def _build_program(T, D, H_per_core, hd):
    import concourse.mybir as mybir
    import concourse.tile as tile
    from concourse import bacc

    f32 = mybir.dt.float32
    at_dt = getattr(mybir.dt, ATTN_DTYPE)
    P = 128
    QT = 512   # q tile width (free dim of score/PV matmuls)
    SC = 128   # s chunk depth (contraction of PV, partitions of scoresT)
    K_D = D // P
    QK_ROWS = 2 * H_per_core * hd
    V_COLS = H_per_core * hd
    C = H_per_core * hd
    n_qt = T // QT
    n_pairs = H_per_core // 2

    nc = bacc.Bacc("TRN2", target_bir_lowering=False, debug=False,
                   num_devices=N_CORES)

    xt_d = nc.declare_dram_parameter("xt", [D, T], at_dt, isOutput=False)
    wqkt_d = nc.declare_dram_parameter("wqkt", [D, QK_ROWS], at_dt, isOutput=False)
    wvt_d = nc.declare_dram_parameter("wvt", [D, V_COLS], at_dt, isOutput=False)
    wot_d = nc.declare_dram_parameter("wot", [C, D], at_dt, isOutput=False)
    masks_d = nc.declare_dram_parameter("masks", [P, 4 * 2 * QT], at_dt,
                                        isOutput=False)
    yt_d = nc.declare_dram_parameter("yt", [D, T], f32, isOutput=True)

    xt_t = xt_d.rearrange("(a p) t -> a p t", p=P)
    wqkt_t = wqkt_d.rearrange("(a p) m -> a p m", p=P)
    wvt_t = wvt_d.rearrange("(a p) m -> a p m", p=P)
    wot_t = wot_d.rearrange("(a p) m -> a p m", p=P)
    yt_t = yt_d.rearrange("(a p) t -> a p t", p=P)

    with nc.allow_low_precision(reason="bf16/f32r compute is intentional"), \
            tile.TileContext(nc) as tc:
        with (
            tc.tile_pool(name="persist", bufs=1) as persist,
            tc.tile_pool(name="xw", bufs=1) as xw,
            tc.tile_pool(name="aps_pool", bufs=2, space="PSUM") as aps_pool,
            tc.tile_pool(name="sc_ps", bufs=2, space="PSUM") as sc_ps,
            tc.tile_pool(name="pv_ps", bufs=1, space="PSUM") as pv_ps,
            tc.tile_pool(name="u_sb", bufs=6) as u_pool,
            tc.tile_pool(name="norm", bufs=1) as norm_pool,
            tc.tile_pool(name="y_sb", bufs=2) as y_pool,
        ):
            # ---- input DMAs ----
            xt = [xw.tile([P, T], at_dt, name=f"xt{i}", tag=f"xt{i}")
                  for i in range(K_D)]
            wqkt = [xw.tile([P, QK_ROWS], at_dt, name=f"wqkt{i}", tag=f"wqkt{i}")
                    for i in range(K_D)]
            wvt = [xw.tile([P, V_COLS], at_dt, name=f"wvt{i}", tag=f"wvt{i}")
                   for i in range(K_D)]
            for i in range(K_D):
                nc.sync.dma_start(out=wqkt[i][:], in_=wqkt_t[i])
            # xt arrives in T-chunks, n-major, so the first QKV psum group can
            # start as soon as the weights + 1/4 of xt have landed
            for i in range(K_D):
                nc.sync.dma_start(out=xt[i][:, 0:512], in_=xt_t[i][:, 0:512])
            for i in range(K_D):
                nc.sync.dma_start(out=wvt[i][:], in_=wvt_t[i])
            for n in range(1, T // 512):
                for i in range(K_D):
                    nc.sync.dma_start(out=xt[i][:, n * 512:(n + 1) * 512],
                                      in_=xt_t[i][:, n * 512:(n + 1) * 512])
            masks = persist.tile([P, 8 * QT], at_dt, name="masks", tag="masks")
            nc.sync.dma_start(out=masks[:], in_=masks_d[:])
            wot = [persist.tile([P, D], at_dt, name=f"wot{i}", tag=f"wot{i}")
                   for i in range(C // P)]
            for i in range(C // P):
                nc.sync.dma_start(out=wot[i][:], in_=wot_t[i])

            ones_f32 = persist.tile([P, hd], f32, name="ones_f32", tag="ones_f32")
            nc.vector.memset(ones_f32[:], 1.0)

            # ---- phase A: qkT (transposed) + v (natural), n-chunk-major ----
            qkt = [persist.tile([P, T], at_dt, name=f"qkt{m}", tag=f"qkt{m}")
                   for m in range(QK_ROWS // P)]
            v_pad = [persist.tile([P, H_per_core * (hd + 1)], at_dt,
                                  name=f"vp{m}", tag=f"vp{m}")
                     for m in range(T // P)]
            for n in range(T // 512):
                for m in range(QK_ROWS // P):
                    ps = aps_pool.tile([P, 512], f32, name="qkps", tag="aps")
                    for k in range(K_D):
                        nc.tensor.matmul(
                            ps[:],
                            lhsT=wqkt[k][:, m * P:(m + 1) * P],
                            rhs=xt[k][:, n * 512:(n + 1) * 512],
                            start=(k == 0), stop=(k == K_D - 1),
                        )
                    nc.vector.tensor_copy(qkt[m][:, n * 512:(n + 1) * 512], ps[:])
                for m in range(4 * n, 4 * n + 4):
                    ones_cols = v_pad[m].rearrange(
                        "p (h x) -> p h x", x=hd + 1)[:, :, hd:]
                    nc.vector.tensor_copy(
                        ones_cols,
                        ones_f32[:, 0:H_per_core].rearrange("p (h x) -> p h x", x=1))
                    ps = aps_pool.tile([P, V_COLS], f32, name="vps", tag="aps")
                    for k in range(K_D):
                        nc.tensor.matmul(
                            ps[:],
                            lhsT=xt[k][:, m * P:(m + 1) * P],
                            rhs=wvt[k][:],
                            start=(k == 0), stop=(k == K_D - 1),
                        )
                    for h in range(H_per_core):
                        nc.vector.tensor_copy(
                            v_pad[m][:, h * (hd + 1):h * (hd + 1) + hd],
                            ps[:, h * hd:(h + 1) * hd],
                        )

            # ---- phase B: attention (g-outer) + interleaved out-projection ----
            attn = [persist.tile([P, T], at_dt, name=f"attn{p}", tag=f"attn{p}")
                    for p in range(n_pairs)]
            for g in range(n_qt):
                for p in range(n_pairs):
                    qT = qkt[p]        # heads (2p, 2p+1) on partitions 0:64, 64:128
                    kT = qkt[n_pairs + p]
                    q0 = g * QT
                    n_sc = (q0 + QT) // SC
                    pv = [pv_ps.tile([hd + 1, QT], f32, name=f"pv{e}", tag=f"pv{e}")
                          for e in range(2)]
                    for j in range(n_sc):
                        s0 = j * SC
                        sc = sc_ps.tile([P, 2 * QT], f32, name="sc", tag="sc")
                        for e in range(2):
                            lo, hi = e * 64, e * 64 + 64
                            nc.tensor.matmul(
                                sc[:, e * QT:(e + 1) * QT],
                                lhsT=kT[lo:hi, s0:s0 + SC],
                                rhs=qT[lo:hi, q0:q0 + QT],
                                start=True, stop=True,
                            )
                        u = u_pool.tile([P, 2 * QT], at_dt, name="u", tag="u")
                        nc.scalar.activation(
                            u[:], sc[:], mybir.ActivationFunctionType.Exp,
                            scale=1.0 / math.sqrt(hd),
                        )
                        r = (s0 - q0) // SC
                        if r >= 0:  # diagonal chunk: apply causal mask (both heads)
                            nc.vector.tensor_mul(
                                u[:], u[:], masks[:, r * 2 * QT:(r + 1) * 2 * QT])
                        for e in range(2):
                            h = 2 * p + e
                            nc.tensor.matmul(
                                pv[e][:],
                                lhsT=v_pad[j][:, h * (hd + 1):(h + 1) * (hd + 1)],
                                rhs=u[:, e * QT:(e + 1) * QT],
                                start=(j == 0), stop=(j == n_sc - 1),
                            )
                    reps = []
                    for e in range(2):
                        # evict pv bank: unnormalized outT + Z row
                        nc.vector.tensor_copy(
                            attn[p][e * hd:(e + 1) * hd, q0:q0 + QT],
                            pv[e][0:hd, :])
                        zrow = norm_pool.tile([1, QT], f32, name="zrow",
                                              tag=f"zrow{e}", bufs=4)
                        nc.vector.tensor_copy(zrow[0:1, :], pv[e][hd:hd + 1, :])
                        rrow = norm_pool.tile([1, QT], f32, name="rrow",
                                              tag=f"rrow{e}", bufs=4)
                        nc.vector.reciprocal_approx_fast(
                            out=rrow[0:1, :], in_=zrow[0:1, :])
                        rep = norm_pool.tile([P, QT], f32, name="rep",
                                             tag=f"rep{e}", bufs=4)
                        nc.gpsimd.partition_broadcast(rep[:], rrow[0:1, :])
                        reps.append(rep)
                    for e in range(2):
                        sl = attn[p][e * hd:(e + 1) * hd, q0:q0 + QT]
                        nc.vector.tensor_mul(
                            sl, sl, reps[e][e * hd:(e + 1) * hd, :])

                # out-projection for this q-tile (t columns g*QT..): both pairs
                # of heads are normalized now, so contract all C rows
                for m in range(D // P):
                    ps = aps_pool.tile([P, QT], f32, name="yps", tag="aps")
                    for k in range(C // P):
                        nc.tensor.matmul(
                            ps[:],
                            lhsT=wot[k][:, m * P:(m + 1) * P],
                            rhs=attn[k][:, g * QT:(g + 1) * QT],
                            start=(k == 0), stop=(k == C // P - 1),
                        )
                    y = y_pool.tile([P, QT], f32, name="y", tag="y", bufs=4)
                    nc.vector.tensor_copy(y[:], ps[:])
                    nc.sync.dma_start(
                        out=yt_t[m][:, g * QT:(g + 1) * QT], in_=y[:])

    nc.finalize()
    return nc


def _make_masks(QT=512, SC=128):
    """Doubled causal masks: [128, 4*2*QT]; block r holds the mask for
    relative offset r twice side by side (head A | head B)."""
    i = np.arange(SC)[:, None]
    j = np.arange(QT)[None, :]
    blocks = []
    for r in range(4):
        m = (i + r * SC <= j).astype(np.float32)
        blocks += [m, m]
    return np.concatenate(blocks, axis=1)


def _cast(arr, dtype_name):
    if dtype_name == "bfloat16":
        import ml_dtypes
        return np.ascontiguousarray(arr.astype(ml_dtypes.bfloat16))
    return np.ascontiguousarray(arr.astype(np.float32))


def kernel(x, Wqkv, Wout, a, b, dt):
    from concourse.bass_utils import run_bass_kernel_spmd

    x = np.asarray(x, dtype=np.float32)
    Wqkv = np.asarray(Wqkv, dtype=np.float32)
    Wout = np.asarray(Wout, dtype=np.float32)
    B, T, D = x.shape
    H, hd = N_HEADS, HEAD_DIM
    hpc = HEADS_PER_CORE
    cores_per_batch = H // hpc
    f0 = _fhn_scale(a, b, dt)

    key = (T, D, hpc, hd)
    if key not in _PROGRAM_CACHE:
        _PROGRAM_CACHE[key] = _build_program(*key)
    nc = _PROGRAM_CACHE[key]

    masks = _cast(_make_masks(), ATTN_DTYPE)
    in_maps = []
    for c in range(N_CORES):
        bi = c // cores_per_batch
        heads = range((c % cores_per_batch) * hpc, (c % cores_per_batch) * hpc + hpc)
        q_rows = np.concatenate([np.arange(h * hd, (h + 1) * hd) for h in heads])
        xt = _cast(x[bi].T, ATTN_DTYPE)                          # (D, T)
        wqk = np.concatenate([Wqkv[q_rows], Wqkv[D + q_rows]], axis=0)
        wqkt = _cast(wqk.T, ATTN_DTYPE)                          # (D, 2*hpc*hd)
        wvt = _cast(Wqkv[2 * D + q_rows].T, ATTN_DTYPE)          # (D, hpc*hd)
        wo = (Wout[:, q_rows].astype(np.float64) * f0).astype(np.float32)
        wot = _cast(wo.T, ATTN_DTYPE)                            # (hpc*hd, D)
        in_maps.append({"xt": xt, "wqkt": wqkt, "wvt": wvt, "wot": wot,
                        "masks": masks})

    trace_dir = os.environ.get("KERNEL_TRACE", "")
    kwargs = {}
    if trace_dir:
        os.makedirs(trace_dir, exist_ok=True)
        kwargs = {"trace": True, "tmpdir": trace_dir}
    res = run_bass_kernel_spmd(nc, in_maps, list(range(N_CORES)), **kwargs)
    LAST_RUN["exec_time_ns"] = res.exec_time_ns
    LAST_RUN["profile_json"] = res.profile_json

    out = np.zeros((B, T, D), dtype=np.float32)
    for bi in range(B):
        acc = np.zeros((D, T), dtype=np.float32)
        for c in range(bi * cores_per_batch, (bi + 1) * cores_per_batch):
            acc += res.results[c]["yt"]
        out[bi] = acc.T
    return out


# revision 28
# speedup vs baseline: 1.6472x; 1.0172x over previous
# Trainium2 Bass kernel for nn_FHNTritonAttention: causal attention with an
# FHN (FitzHugh-Nagumo) gate on the attention probabilities.
#
# Math note that shapes the whole kernel: attn_energy = softmax(scores).sum(-1)
# is ~1.0 for every row (softmax rows sum to 1), so stimulus_normed == 1,
# threshold_gate == sigmoid(5), and the FHN recurrence collapses to one
# constant per run. The gate multiplies each probability row by a constant c
# and the subsequent renormalization divides it back out:
#   p'' = p*c / (c*S + 1e-8) = p / (S + 1e-8/c),  S = row sum ~= 1.
# So the entire FHN block reduces to scaling the output by
# f0 = 1/(1 + 1e-8/c0), computed on host from (a, b, dt) and folded into Wout.
# The deviations this ignores are O(1e-7) relative — far below fp32 matmul
# noise of the reference itself.
#
# Device kernel (SPMD over 8 cores; core = (batch, 4-head group)):
#   phase A: qkT = Wqk_slice @ x.T   (transposed layout: feature on partitions)
#            v_nat = x @ Wv_slice.T  (natural layout, + ones column for Z)
#   phase B: per head PAIR (two heads side by side in one 128-partition tile),
#            per 512-wide q tile, per 128-deep s chunk:
#            scoresT = k q^T (s on partitions) for both heads into one
#            [128, 1024] psum (2 banks), one exp -> U bf16, one causal-mask
#            multiply on diagonal chunks (mask doubled per head),
#            PV: [v | 1]^T @ U accumulates outT and the softmax denominator Z
#            in one matmul per head. pv psum is evicted by two quick copies
#            (outT -> attn tile, Z row -> zbuf); reciprocals are batched per
#            pair (one DVE reciprocal for 8 rows), replication of 1/Z across
#            64 partitions via a 1-partition matmul, normalize multiplies
#            in-place against the replication PSUM.
#   phase C: yT_partial = Wout_slice @ attn_outT  -> DMA out; host sums the 4
#            partial products per batch and transposes back.
#
# Matmuls run in bf16 (inputs pre-cast on host; fp32 PSUM accumulation), the
# 1/Z path in float32r.

import math
import os

import numpy as np

N_HEADS = 16
HEAD_DIM = 64
THRESHOLD = 0.5
TAU = 12.5
N_FHN_STEPS = 2

N_CORES = 8
HEADS_PER_CORE = 4  # cores 0-3 -> batch 0, cores 4-7 -> batch 1

ATTN_DTYPE = os.environ.get("KERNEL_ATTN_DTYPE", "bfloat16")

LAST_RUN = {}  # filled with exec_time_ns etc. when KERNEL_TRACE is set

_PROGRAM_CACHE = {}


def _fhn_scale(a, b, dt):
    """Host-side replica of the reference's gate math at attn_energy == 1."""
    a = float(a)
    b = float(b)
    dt = float(dt)
    sig5 = 1.0 / (1.0 + math.exp(-(1.0 - THRESHOLD) * 10.0))
    i0 = 1.0 * (0.1 + 0.9 * sig5)
    v = 0.0
    w = 0.0
    for _ in range(N_FHN_STEPS):
        v = v + dt * (v - v**3 / 3.0 - w + i0)
        w = (w + (dt / TAU) * (v + a)) / (1.0 + (dt / TAU) * b)
    gate = 1.0 / (1.0 + math.exp(-v))
    c0 = 0.5 + 0.5 * gate
    return c0 / (c0 + 1e-8)


def _build_program(T, D, H_per_core, hd):
    import concourse.mybir as mybir
    import concourse.tile as tile
    from concourse import bacc

    f32 = mybir.dt.float32
    at_dt = getattr(mybir.dt, ATTN_DTYPE)
    P = 128
    QT = 512   # q tile width (free dim of score/PV matmuls)
    SC = 128   # s chunk depth (contraction of PV, partitions of scoresT)
    K_D = D // P
    QK_ROWS = 2 * H_per_core * hd
    V_COLS = H_per_core * hd
    C = H_per_core * hd
    n_qt = T // QT
    n_pairs = H_per_core // 2

    nc = bacc.Bacc("TRN2", target_bir_lowering=False, debug=False,
                   num_devices=N_CORES)

    xt_d = nc.declare_dram_parameter("xt", [D, T], at_dt, isOutput=False)
    wqkt_d = nc.declare_dram_parameter("wqkt", [D, QK_ROWS], at_dt, isOutput=False)
    wvt_d = nc.declare_dram_parameter("wvt", [D, V_COLS], at_dt, isOutput=False)
    wot_d = nc.declare_dram_parameter("wot", [C, D], at_dt, isOutput=False)
    masks_d = nc.declare_dram_parameter("masks", [P, 4 * 2 * QT], at_dt,
                                        isOutput=False)
    yt_d = nc.declare_dram_parameter("yt", [D, T], f32, isOutput=True)

    xt_t = xt_d.rearrange("(a p) t -> a p t", p=P)
    wqkt_t = wqkt_d.rearrange("(a p) m -> a p m", p=P)
    wvt_t = wvt_d.rearrange("(a p) m -> a p m", p=P)
    wot_t = wot_d.rearrange("(a p) m -> a p m", p=P)
    yt_t = yt_d.rearrange("(a p) t -> a p t", p=P)

    with nc.allow_low_precision(reason="bf16/f32r compute is intentional"), \
            tile.TileContext(nc) as tc:
        with (
            tc.tile_pool(name="persist", bufs=1) as persist,
            tc.tile_pool(name="xw", bufs=1) as xw,
            tc.tile_pool(name="aps_pool", bufs=2, space="PSUM") as aps_pool,
            tc.tile_pool(name="sc_ps", bufs=2, space="PSUM") as sc_ps,
            tc.tile_pool(name="pv_ps", bufs=1, space="PSUM") as pv_ps,
            tc.tile_pool(name="u_sb", bufs=6) as u_pool,
            tc.tile_pool(name="norm", bufs=1) as norm_pool,
            tc.tile_pool(name="y_sb", bufs=2) as y_pool,
        ):
            # ---- input DMAs ----
            xt = [xw.tile([P, T], at_dt, name=f"xt{i}", tag=f"xt{i}")
                  for i in range(K_D)]
            wqkt = [xw.tile([P, QK_ROWS], at_dt, name=f"wqkt{i}", tag=f"wqkt{i}")
                    for i in range(K_D)]
            wvt = [xw.tile([P, V_COLS], at_dt, name=f"wvt{i}", tag=f"wvt{i}")
                   for i in range(K_D)]
            for i in range(K_D):
                nc.sync.dma_start(out=wqkt[i][:], in_=wqkt_t[i])
            # xt arrives in T-chunks, n-major, so the first QKV psum group can
            # start as soon as the weights + 1/4 of xt have landed
            for i in range(K_D):
                nc.sync.dma_start(out=xt[i][:, 0:512], in_=xt_t[i][:, 0:512])
            for i in range(K_D):
                nc.sync.dma_start(out=wvt[i][:], in_=wvt_t[i])
            for n in range(1, T // 512):
                for i in range(K_D):
                    nc.sync.dma_start(out=xt[i][:, n * 512:(n + 1) * 512],
                                      in_=xt_t[i][:, n * 512:(n + 1) * 512])
            masks = persist.tile([P, 8 * QT], at_dt, name="masks", tag="masks")
            nc.sync.dma_start(out=masks[:], in_=masks_d[:])
            wot = [persist.tile([P, D], at_dt, name=f"wot{i}", tag=f"wot{i}")
                   for i in range(C // P)]
            for i in range(C // P):
                nc.sync.dma_start(out=wot[i][:], in_=wot_t[i])

            ones_f32 = persist.tile([P, hd], f32, name="ones_f32", tag="ones_f32")
            nc.vector.memset(ones_f32[:], 1.0)

            # ---- phase A: qkT (transposed) + v (natural), n-chunk-major ----
            qkt = [persist.tile([P, T], at_dt, name=f"qkt{m}", tag=f"qkt{m}")
                   for m in range(QK_ROWS // P)]
            v_pad = [persist.tile([P, H_per_core * (hd + 1)], at_dt,
                                  name=f"vp{m}", tag=f"vp{m}")
                     for m in range(T // P)]
            for n in range(T // 512):
                for m in range(QK_ROWS // P):
                    ps = aps_pool.tile([P, 512], f32, name="qkps", tag="aps")
                    for k in range(K_D):
                        nc.tensor.matmul(
                            ps[:],
                            lhsT=wqkt[k][:, m * P:(m + 1) * P],
                            rhs=xt[k][:, n * 512:(n + 1) * 512],
                            start=(k == 0), stop=(k == K_D - 1),
                        )
                    nc.vector.tensor_copy(qkt[m][:, n * 512:(n + 1) * 512], ps[:])
                for m in range(4 * n, 4 * n + 4):
                    ones_cols = v_pad[m].rearrange(
                        "p (h x) -> p h x", x=hd + 1)[:, :, hd:]
                    nc.vector.tensor_copy(
                        ones_cols,
                        ones_f32[:, 0:H_per_core].rearrange("p (h x) -> p h x", x=1))
                    ps = aps_pool.tile([P, V_COLS], f32, name="vps", tag="aps")
                    for k in range(K_D):
                        nc.tensor.matmul(
                            ps[:],
                            lhsT=xt[k][:, m * P:(m + 1) * P],
                            rhs=wvt[k][:],
                            start=(k == 0), stop=(k == K_D - 1),
                        )
                    for h in range(H_per_core):
                        nc.vector.tensor_copy(
                            v_pad[m][:, h * (hd + 1):h * (hd + 1) + hd],
                            ps[:, h * hd:(h + 1) * hd],
                        )

            # ---- phase B: attention (g-outer) + interleaved out-projection ----
            attn = [persist.tile([P, T], at_dt, name=f"attn{p}", tag=f"attn{p}")
                    for p in range(n_pairs)]
            for g in range(n_qt):
                for p in range(n_pairs):
                    qT = qkt[p]        # heads (2p, 2p+1) on partitions 0:64, 64:128
                    kT = qkt[n_pairs + p]
                    q0 = g * QT
                    n_sc = (q0 + QT) // SC
                    pv = [pv_ps.tile([hd + 1, QT], f32, name=f"pv{e}", tag=f"pv{e}")
                          for e in range(2)]
                    for j in range(n_sc):
                        s0 = j * SC
                        sc = sc_ps.tile([P, 2 * QT], f32, name="sc", tag="sc")
                        for e in range(2):
                            lo, hi = e * 64, e * 64 + 64
                            nc.tensor.matmul(
                                sc[:, e * QT:(e + 1) * QT],
                                lhsT=kT[lo:hi, s0:s0 + SC],
                                rhs=qT[lo:hi, q0:q0 + QT],
                                start=True, stop=True,
                            )
                        u = u_pool.tile([P, 2 * QT], at_dt, name="u", tag="u")
                        r = (s0 - q0) // SC
                        w0 = max(r, 0) * SC  # leading fully-masked columns
                        if w0 == 0:
                            nc.scalar.activation(
                                u[:], sc[:], mybir.ActivationFunctionType.Exp,
                                scale=1.0 / math.sqrt(hd),
                            )
                        else:
                            for e in range(2):
                                off = e * QT
                                nc.scalar.activation(
                                    u[:, off + w0:off + QT],
                                    sc[:, off + w0:off + QT],
                                    mybir.ActivationFunctionType.Exp,
                                    scale=1.0 / math.sqrt(hd),
                                )
                        if r >= 0:  # mask the 128-wide triangle block per head
                            for e in range(2):
                                tri = r * 2 * QT + e * QT + w0
                                nc.vector.tensor_mul(
                                    u[:, e * QT + w0:e * QT + w0 + SC],
                                    u[:, e * QT + w0:e * QT + w0 + SC],
                                    masks[:, tri:tri + SC])
                        for e in range(2):
                            h = 2 * p + e
                            nc.tensor.matmul(
                                pv[e][:, w0:QT],
                                lhsT=v_pad[j][:, h * (hd + 1):(h + 1) * (hd + 1)],
                                rhs=u[:, e * QT + w0:(e + 1) * QT],
                                start=(j == 0), stop=(j == n_sc - 1),
                            )
                    reps = []
                    for e in range(2):
                        # evict pv bank: unnormalized outT + Z row
                        nc.vector.tensor_copy(
                            attn[p][e * hd:(e + 1) * hd, q0:q0 + QT],
                            pv[e][0:hd, :])
                        zrow = norm_pool.tile([1, QT], f32, name="zrow",
                                              tag=f"zrow{e}", bufs=4)
                        nc.vector.tensor_copy(zrow[0:1, :], pv[e][hd:hd + 1, :])
                        rrow = norm_pool.tile([1, QT], f32, name="rrow",
                                              tag=f"rrow{e}", bufs=4)
                        nc.vector.reciprocal_approx_fast(
                            out=rrow[0:1, :], in_=zrow[0:1, :])
                        rep = norm_pool.tile([P, QT], f32, name="rep",
                                             tag=f"rep{e}", bufs=4)
                        nc.gpsimd.partition_broadcast(rep[:], rrow[0:1, :])
                        reps.append(rep)
                    for e in range(2):
                        sl = attn[p][e * hd:(e + 1) * hd, q0:q0 + QT]
                        nc.vector.tensor_mul(
                            sl, sl, reps[e][e * hd:(e + 1) * hd, :])

                # out-projection for this q-tile (t columns g*QT..): both pairs
                # of heads are normalized now, so contract all C rows
                for m in range(D // P):
                    ps = aps_pool.tile([P, QT], f32, name="yps", tag="aps")
                    for k in range(C // P):
                        nc.tensor.matmul(
                            ps[:],
                            lhsT=wot[k][:, m * P:(m + 1) * P],
                            rhs=attn[k][:, g * QT:(g + 1) * QT],
                            start=(k == 0), stop=(k == C // P - 1),
                        )
                    y = y_pool.tile([P, QT], f32, name="y", tag="y", bufs=4)
                    nc.vector.tensor_copy(y[:], ps[:])
                    nc.sync.dma_start(
                        out=yt_t[m][:, g * QT:(g + 1) * QT], in_=y[:])

    nc.finalize()
    return nc


def _make_masks(QT=512, SC=128):
    """Doubled causal masks: [128, 4*2*QT]; block r holds the mask for
    relative offset r twice side by side (head A | head B)."""
    i = np.arange(SC)[:, None]
    j = np.arange(QT)[None, :]
    blocks = []
    for r in range(4):
        m = (i + r * SC <= j).astype(np.float32)
        blocks += [m, m]
    return np.concatenate(blocks, axis=1)


def _cast(arr, dtype_name):
    if dtype_name == "bfloat16":
        import ml_dtypes
        return np.ascontiguousarray(arr.astype(ml_dtypes.bfloat16))
    return np.ascontiguousarray(arr.astype(np.float32))


def kernel(x, Wqkv, Wout, a, b, dt):
    from concourse.bass_utils import run_bass_kernel_spmd

    x = np.asarray(x, dtype=np.float32)
    Wqkv = np.asarray(Wqkv, dtype=np.float32)
    Wout = np.asarray(Wout, dtype=np.float32)
    B, T, D = x.shape
    H, hd = N_HEADS, HEAD_DIM
    hpc = HEADS_PER_CORE
    cores_per_batch = H // hpc
    f0 = _fhn_scale(a, b, dt)

    key = (T, D, hpc, hd)
    if key not in _PROGRAM_CACHE:
        _PROGRAM_CACHE[key] = _build_program(*key)
    nc = _PROGRAM_CACHE[key]

    masks = _cast(_make_masks(), ATTN_DTYPE)
    in_maps = []
    for c in range(N_CORES):
        bi = c // cores_per_batch
        heads = range((c % cores_per_batch) * hpc, (c % cores_per_batch) * hpc + hpc)
        q_rows = np.concatenate([np.arange(h * hd, (h + 1) * hd) for h in heads])
        xt = _cast(x[bi].T, ATTN_DTYPE)                          # (D, T)
        wqk = np.concatenate([Wqkv[q_rows], Wqkv[D + q_rows]], axis=0)
        wqkt = _cast(wqk.T, ATTN_DTYPE)                          # (D, 2*hpc*hd)
        wvt = _cast(Wqkv[2 * D + q_rows].T, ATTN_DTYPE)          # (D, hpc*hd)
        wo = (Wout[:, q_rows].astype(np.float64) * f0).astype(np.float32)
        wot = _cast(wo.T, ATTN_DTYPE)                            # (hpc*hd, D)
        in_maps.append({"xt": xt, "wqkt": wqkt, "wvt": wvt, "wot": wot,
                        "masks": masks})

    trace_dir = os.environ.get("KERNEL_TRACE", "")
    kwargs = {}
    if trace_dir:
        os.makedirs(trace_dir, exist_ok=True)
        kwargs = {"trace": True, "tmpdir": trace_dir}
    res = run_bass_kernel_spmd(nc, in_maps, list(range(N_CORES)), **kwargs)
    LAST_RUN["exec_time_ns"] = res.exec_time_ns
    LAST_RUN["profile_json"] = res.profile_json

    out = np.zeros((B, T, D), dtype=np.float32)
    for bi in range(B):
        acc = np.zeros((D, T), dtype=np.float32)
        for c in range(bi * cores_per_batch, (bi + 1) * cores_per_batch):
            acc += res.results[c]["yt"]
        out[bi] = acc.T
    return out
